# revision 12
# baseline (speedup 1.0000x reference)
"""Trainium2 Bass kernel for nn_BartAttention_66786741453241 (8 NeuronCores).

Reference (bugs preserved): no softmax — raw attention scores are used for the
AV matmul, and q is scaled by dh**-0.5 with scores further divided by sqrt(dh),
net 1/dh. The whole computation is therefore LINEAR in V, so we reassociate
    (Q K^T / 64) V  ==  Q (K^T V) / 64
which collapses the [T,T] score matrices into per-head [64,64] K^T V matrices
(~32x fewer attention FLOPs, exact in infinite precision).

Sharding: tensor-parallel by (batch, head-group) — core i handles batch i//4
and heads 4*(i%4) .. 4*(i%4)+4 for ALL 2048 tokens of that batch:
  - fused k|v projection (concatenated weight slice) -> per-head K^T V is
    complete locally: NO collective anywhere,
  - qT projection for its 4 heads, per-head OT_h = lhsT(KTV_h).T @ qT_h,
  - partial out^T = WoT-slice proj of O^T (bf16), DMA'd out per core.
The host sums the 4 partials per batch and adds bo — that host-side reduce is
the unshard step for the out_proj input-dim sharding (the "all-reduce after
out_proj" of the standard tensor-parallel recipe).
All matmuls run in bf16 (fp32 PSUM accumulate); measured end-to-end relative
error vs the f32 reference ~5e-3 (gate 2e-2).
"""

import os
import sys
import types

import numpy as np
import ml_dtypes

import concourse.bass as bass
import concourse.bacc as bacc
import concourse.mybir as mybir
import concourse.tile as tile
from concourse.bass_utils import run_bass_kernel_spmd

BF16 = mybir.dt.bfloat16
F32 = mybir.dt.float32
NPBF16 = ml_dtypes.bfloat16

E = 1024        # embed dim
H = 16          # heads
DH = 64         # head dim
B, T = 2, 2048
NC = 8          # cores
P = 128
KC = E // P     # 8 contraction chunks for the in-projections
HPC = 4         # heads per core
EH = HPC * DH   # 256: per-core q/k/v feature width
TG = T // 512   # 4 moving-dim groups of 512 tokens
TTC = T // P    # 16 token chunks per core
Ident = mybir.ActivationFunctionType.Identity
OUT_BF16 = True  # partial out^T in bf16 (halves the output DMA)


def _install_axon_profile_hook():
    """Make trace=True usable under axon: register the NTFF hook that the
    staged antenv lacks, and neuter artifact upload (no bucket here). Safe
    no-op when pieces are missing."""
    try:
        import concourse.bass_utils as bu
        bu.upload_artifacts = lambda tmpdir: "local://" + tmpdir
    except Exception:
        pass
    if "antenv.axon_hooks" in sys.modules:
        return
    hook = None
    try:
        from trn_agent_boot.trn_boot import _ntff_profile_via_ctypes
        so = "/opt/axon/libaxon_pjrt.so"
        if os.path.exists(so):
            hook = _ntff_profile_via_ctypes(so)
    except Exception:
        hook = None
    mod = types.ModuleType("antenv.axon_hooks")
    mod.get_axon_ntff_profile_hook = lambda: hook
    mod.set_axon_ntff_profile_hook = lambda h: None
    sys.modules["antenv.axon_hooks"] = mod


def build():
    """Build + compile the per-core SPMD graph (identical on all 8 cores)."""
    nc = bacc.Bacc("TRN2", target_bir_lowering=False, debug=False, num_devices=NC)

    out_dt = BF16 if OUT_BF16 else F32
    hsT = nc.dram_tensor("hsT", [E, T], BF16, kind="ExternalInput")       # 4 MB
    wkvt = nc.dram_tensor("wkvt", [E, 2 * EH], BF16, kind="ExternalInput")  # 1 MB
    wqt = nc.dram_tensor("wqt", [E, EH], BF16, kind="ExternalInput")      # 0.5 MB
    wot = nc.dram_tensor("wot", [EH, E], BF16, kind="ExternalInput")      # 0.5 MB
    bkvb = nc.dram_tensor("bkvb", [P, 2 * EH], F32, kind="ExternalInput")  # pre-tiled
    bq_t = nc.dram_tensor("bq_t", [P, 16], F32, kind="ExternalInput")  # cols 0-1 used
    outT = nc.dram_tensor("outT", [E, T], out_dt, kind="ExternalOutput")

    with tile.TileContext(nc) as tc:
        with (
            tc.tile_pool(name="sb", bufs=1) as sb,
            tc.tile_pool(name="stg", bufs=3) as stg,
            tc.tile_pool(name="psA", bufs=6, space="PSUM") as psA,
            tc.tile_pool(name="psB", bufs=2, space="PSUM") as psB,
        ):
            # ---- PE warm-up: dummy matmuls on memset tiles keep the PE's HAM
            # activity window busy during the input-DMA wait so the real
            # stream starts at 2.4 GHz instead of ramping from 1.2.
            dum_w = sb.tile([P, P], BF16, tag="dum_w")
            nc.gpsimd.memset(dum_w[:], 0.0)
            dum_x = sb.tile([P, 512], BF16, tag="dum_x")
            nc.gpsimd.memset(dum_x[:], 0.0)
            dum_ps = psB.tile([P, 512], F32, tag="psB")
            for _ in range(22):
                nc.tensor.matmul(dum_ps[:], dum_w[:], dum_x[:], start=True, stop=True)
            dum_out = sb.tile([P, 4], BF16, tag="dum_out")
            nc.vector.tensor_copy(dum_out[:], dum_ps[:, 0:4])

            # ---- loads; wkv/hs in quarter tiles so the first chunks land
            # fast and the projection stream starts early
            NQ = 4
            wkv_q = [
                sb.tile([P, 2 * 2 * EH], BF16, tag=f"wkv{i}", name=f"wkv{i}")
                for i in range(NQ)
            ]
            hs_q = [
                sb.tile([P, 2 * T], BF16, tag=f"hs{i}", name=f"hs{i}")
                for i in range(NQ)
            ]
            for i in range(NQ):
                nc.sync.dma_start(
                    wkv_q[i][:].rearrange("p (c n) -> p c n", c=2),
                    wkvt.ap().rearrange("(c p) n -> p c n", p=P)[:, 2 * i:2 * i + 2, :],
                )
                nc.sync.dma_start(
                    hs_q[i][:].rearrange("p (c t) -> p c t", c=2),
                    hsT.ap().rearrange("(c p) t -> p c t", p=P)[:, 2 * i:2 * i + 2, :],
                )
            bkv_sb = sb.tile([P, 2 * EH], F32, tag="bkv")
            nc.gpsimd.dma_start(bkv_sb[:], bkvb[:, :])
            wq_big = sb.tile([P, KC * EH], BF16, tag="wq")
            nc.gpsimd.dma_start(
                wq_big[:].rearrange("p (c n) -> p c n", c=KC),
                wqt.ap().rearrange("(c p) n -> p c n", p=P),
            )
            bq_sb = sb.tile([P, 16], F32, tag="bq")
            nc.gpsimd.dma_start(bq_sb[:], bq_t[:, :])
            wo_sb = [sb.tile([P, E], BF16, tag=f"wo{c}", name=f"wo{c}") for c in range(2)]
            for c in range(2):
                nc.gpsimd.dma_start(wo_sb[c][:], wot[c * P:(c + 1) * P, :])

            def hs_c(c):
                return hs_q[c // 2][:, (c % 2) * T:(c % 2 + 1) * T]

            def wkv_c(c):
                return wkv_q[c // 2][:, (c % 2) * 2 * EH:(c % 2 + 1) * 2 * EH]

            # ---- fused k|v projection: [128 tokens, k(4 heads)|v(4 heads)]
            kv_sb = [
                sb.tile([P, 2 * EH], BF16, tag=f"kv{tt}", name=f"kv{tt}")
                for tt in range(TTC)
            ]
            for tt in range(TTC):
                ps = psA.tile([P, 512], F32, tag="psA")
                for c in range(KC):
                    nc.tensor.matmul(
                        ps[:],
                        hs_c(c)[:, tt * P:(tt + 1) * P],
                        wkv_c(c),
                        start=(c == 0),
                        stop=(c == KC - 1),
                    )
                nc.vector.tensor_add(kv_sb[tt][:], ps[:], bkv_sb[:])

            # ---- per-head K^T V (full batch, local: no collective)
            # head pairs stacked on partitions: head 2j+hh at rows hh*64,
            # cols j*64 — bases line up with qT slices in the Q@KTV matmul.
            ktv_bf = sb.tile([P, (HPC // 2) * DH], BF16, tag="ktv_bf")
            for j in range(HPC // 2):
                ps = psB.tile([P, DH], F32, tag="psB")
                for tt in range(TTC):
                    for hh in range(2):
                        # two heads of the pair hit distinct PE column groups
                        # (psum partitions 0-63 / 64-127) and run concurrently
                        h = 2 * j + hh
                        r0 = hh * DH
                        nc.tensor.matmul(
                            ps[r0:r0 + DH, :],
                            kv_sb[tt][:, h * DH:(h + 1) * DH],
                            kv_sb[tt][:, EH + h * DH:EH + (h + 1) * DH],
                            start=(tt == 0),
                            stop=(tt == TTC - 1),
                        )
                nc.vector.tensor_copy(ktv_bf[:, j * DH:(j + 1) * DH], ps[:])

            # ---- qT projection [e_out 256, tokens], bias + 1/64 folded
            q_sb = [
                sb.tile([P, T], BF16, tag=f"q{m}", name=f"q{m}")
                for m in range(EH // P)
            ]
            for m in range(EH // P):
                for tg in range(TG):
                    ps = psA.tile([P, 512], F32, tag="psA")
                    for c in range(KC):
                        nc.tensor.matmul(
                            ps[:],
                            wq_big[:, c * EH + m * P:c * EH + (m + 1) * P],
                            hs_c(c)[:, tg * 512:(tg + 1) * 512],
                            start=(c == 0),
                            stop=(c == KC - 1),
                        )
                    nc.scalar.activation(
                        q_sb[m][:, tg * 512:(tg + 1) * 512], ps[:], Ident,
                        bias=bq_sb[:, m:m + 1], scale=1.0 / 64.0,
                    )

            # ---- O^T per head pair: OT_h[dv, t] = lhsT(KTV_h).T @ qT_h
            oT_sb = [
                sb.tile([P, T], BF16, tag=f"oT{m}", name=f"oT{m}")
                for m in range(EH // P)
            ]
            for j in range(HPC // 2):
                for tg in range(TG):
                    ps = psA.tile([P, 512], F32, tag="psA")
                    for hh in range(2):
                        r0 = hh * DH
                        nc.tensor.matmul(
                            ps[r0:r0 + DH, :],
                            ktv_bf[r0:r0 + DH, j * DH:(j + 1) * DH],
                            q_sb[j][r0:r0 + DH, tg * 512:(tg + 1) * 512],
                            start=True,
                            stop=True,
                        )
                    nc.vector.tensor_copy(oT_sb[j][:, tg * 512:(tg + 1) * 512], ps[:])

            # ---- partial out^T = WoT-slice proj (no bias: host adds bo once)
            for m in range(KC):
                o_stage = stg.tile([P, T], out_dt, tag="ostg")
                for tg in range(TG):
                    ps = psA.tile([P, 512], F32, tag="psA")
                    for c in range(2):
                        nc.tensor.matmul(
                            ps[:],
                            wo_sb[c][:, m * P:(m + 1) * P],
                            oT_sb[c][:, tg * 512:(tg + 1) * 512],
                            start=(c == 0),
                            stop=(c == 1),
                        )
                    if tg % 2 == 0:
                        nc.vector.tensor_copy(o_stage[:, tg * 512:(tg + 1) * 512], ps[:])
                    else:
                        nc.scalar.copy(o_stage[:, tg * 512:(tg + 1) * 512], ps[:])
                nc.sync.dma_start(outT[m * P:(m + 1) * P, :], o_stage[:])

    nc.compile()
    return nc


_NC_CACHE = None


def _get_nc():
    global _NC_CACHE
    if _NC_CACHE is None:
        _install_axon_profile_hook()
        _NC_CACHE = build()
    return _NC_CACHE


def bq_pad(bq_slice):
    """[256] -> padded [128, 16] f32 (cols 0-1 used): contiguous DMA layout."""
    t = np.zeros((P, 16), np.float32)
    t[:, 0:2] = bq_slice.reshape(2, P).T
    return t


def make_in_maps(hidden_states, Wq, bq, Wk, bk, Wv, bv, Wo, bo):
    f32 = np.float32
    hs = np.asarray(hidden_states, f32)
    WqT = np.asarray(Wq, f32).T    # [e_in, e_out]
    WkT = np.asarray(Wk, f32).T
    WvT = np.asarray(Wv, f32).T
    WoT = np.asarray(Wo, f32).T
    bq64 = np.asarray(bq, f32) / 64.0
    bk = np.asarray(bk, f32)
    bv = np.asarray(bv, f32)

    hsT_b = [
        np.ascontiguousarray(hs[b].T).astype(NPBF16) for b in range(B)
    ]
    in_maps = []
    for i in range(NC):
        g, r = divmod(i, HPC)
        sl = slice(r * EH, (r + 1) * EH)
        wkvt = np.concatenate([WkT[:, sl], WvT[:, sl]], axis=1)
        bkv = np.concatenate([bk[sl], bv[sl]])
        in_maps.append({
            "hsT": hsT_b[g],
            "wkvt": np.ascontiguousarray(wkvt).astype(NPBF16),
            "wqt": np.ascontiguousarray(WqT[:, sl]).astype(NPBF16),
            "wot": np.ascontiguousarray(WoT[sl, :]).astype(NPBF16),
            "bkvb": np.ascontiguousarray(np.broadcast_to(bkv, (P, 2 * EH))),
            "bq_t": bq_pad(bq64[sl]),
        })
    return in_maps


def run(inputs, trace=False, **kw):
    """Run on 8 NeuronCores; returns (full_output [B,T,E] f32, BassKernelResults)."""
    nc = _get_nc()
    in_maps = make_in_maps(**inputs)
    res = run_bass_kernel_spmd(nc, in_maps, list(range(NC)), trace=trace, **kw)
    bo = np.asarray(inputs["bo"], np.float32)
    out = np.empty((B, T, E), np.float32)
    for g in range(B):
        acc = res.results[g * HPC]["outT"].astype(np.float32)
        for r in range(1, HPC):
            acc = acc + res.results[g * HPC + r]["outT"].astype(np.float32)
        out[g] = acc.T + bo
    return out, res


def kernel(**inputs):
    out, _ = run(inputs, trace=False)
    return out


# revision 34
# speedup vs baseline: 1.1141x; 1.1141x over previous
"""Trainium2 Bass kernel for nn_BartAttention_66786741453241 (8 NeuronCores).

Reference (bugs preserved): no softmax — raw attention scores are used for the
AV matmul, and q is scaled by dh**-0.5 with scores further divided by sqrt(dh),
net 1/dh. The whole computation is therefore LINEAR in V, so we reassociate
    (Q K^T / 64) V  ==  Q (K^T V) / 64
which collapses the [T,T] score matrices into per-head [64,64] K^T V matrices
(~32x fewer attention FLOPs, exact in infinite precision).

Sharding: tensor-parallel by (batch, head-group) — core i handles batch i//4
and heads 4*(i%4) .. 4*(i%4)+4 for ALL 2048 tokens of that batch:
  - fused k|v projection (concatenated weight slice) -> per-head K^T V is
    complete locally: NO collective anywhere,
  - block-diagonal pair tiles of V^T K feed M_j = blockdiag(KTV) @ WoT_pair,
    so the tail is one matmul family: partial out^T = sum_j M_j^T @ qT_j,
  - qT projection for its 4 heads (bias + the net 1/64 scaling folded in),
  - partial out^T (bf16) DMA'd out per core.
The host sums the 4 partials per batch and adds bo — that host-side reduce is
the unshard step for the out_proj input-dim sharding (the "all-reduce after
out_proj" of the standard tensor-parallel recipe).
Other details: PE warm-up via dummy matmuls during the input-DMA wait (keeps
the HAM clock-gate at 2.4 GHz for the real stream), inputs in consumption-
order quarter tiles, all matmuls bf16 (fp32 PSUM accumulate); end-to-end
relative error vs the f32 reference ~4.7e-3 (gate 2e-2).
"""

import os
import sys
import types

import numpy as np
import ml_dtypes

import concourse.bacc as bacc
import concourse.mybir as mybir
import concourse.tile as tile
from concourse.bass_utils import run_bass_kernel_spmd

BF16 = mybir.dt.bfloat16
F32 = mybir.dt.float32
NPBF16 = ml_dtypes.bfloat16

E = 1024        # embed dim
H = 16          # heads
DH = 64         # head dim
B, T = 2, 2048
NC = 8          # cores
P = 128
KC = E // P     # 8 contraction chunks for the in-projections
HPC = 4         # heads per core
EH = HPC * DH   # 256: per-core q/k/v feature width
TG = T // 512   # 4 moving-dim groups of 512 tokens
TTC = T // P    # 16 token chunks per core
Ident = mybir.ActivationFunctionType.Identity
OUT_BF16 = True  # partial out^T in bf16 (halves the output DMA)


def _install_axon_profile_hook():
    """Make trace=True usable under axon: register the NTFF hook that the
    staged antenv lacks, and neuter artifact upload (no bucket here). Safe
    no-op when pieces are missing."""
    try:
        import concourse.bass_utils as bu
        bu.upload_artifacts = lambda tmpdir: "local://" + tmpdir
    except Exception:
        pass
    if "antenv.axon_hooks" in sys.modules:
        return
    hook = None
    try:
        from trn_agent_boot.trn_boot import _ntff_profile_via_ctypes
        so = "/opt/axon/libaxon_pjrt.so"
        if os.path.exists(so):
            hook = _ntff_profile_via_ctypes(so)
    except Exception:
        hook = None
    mod = types.ModuleType("antenv.axon_hooks")
    mod.get_axon_ntff_profile_hook = lambda: hook
    mod.set_axon_ntff_profile_hook = lambda h: None
    sys.modules["antenv.axon_hooks"] = mod


def build():
    """Build + compile the per-core SPMD graph (identical on all 8 cores)."""
    nc = bacc.Bacc("TRN2", target_bir_lowering=False, debug=False, num_devices=NC)

    out_dt = BF16 if OUT_BF16 else F32
    hsT = nc.dram_tensor("hsT", [E, T], BF16, kind="ExternalInput")       # 4 MB
    wkvt = nc.dram_tensor("wkvt", [E, 2 * EH], BF16, kind="ExternalInput")  # 1 MB
    wqt = nc.dram_tensor("wqt", [E, EH], BF16, kind="ExternalInput")      # 0.5 MB
    wot = nc.dram_tensor("wot", [EH, E], BF16, kind="ExternalInput")      # 0.5 MB
    # pre-tiled biases: cols 0..512 = k|v bias rows, cols 512..514 = bq/64
    bkvb = nc.dram_tensor("bkvb", [P, 2 * EH + 16], F32, kind="ExternalInput")
    outT = nc.dram_tensor("outT", [E, T], out_dt, kind="ExternalOutput")

    with tile.TileContext(nc) as tc:
        with (
            tc.tile_pool(name="sb", bufs=1) as sb,
            tc.tile_pool(name="stg", bufs=3) as stg,
            tc.tile_pool(name="psA", bufs=6, space="PSUM") as psA,
            tc.tile_pool(name="psB", bufs=2, space="PSUM") as psB,
        ):
            # ---- PE warm-up: dummy matmuls on memset tiles keep the PE's HAM
            # activity window busy during the input-DMA wait so the real
            # stream starts at 2.4 GHz instead of ramping from 1.2.
            dum_w = sb.tile([P, P], BF16, tag="dum_w")
            nc.gpsimd.memset(dum_w[:], 0.0)
            dum_x = sb.tile([P, 512], BF16, tag="dum_x")
            nc.gpsimd.memset(dum_x[:], 0.0)
            dum_ps = psB.tile([P, 512], F32, tag="psB")
            for _ in range(15):
                nc.tensor.matmul(dum_ps[:], dum_w[:], dum_x[:], start=True, stop=True)
            dum_out = sb.tile([P, 4], BF16, tag="dum_out")
            nc.vector.tensor_copy(dum_out[:], dum_ps[:, 0:4])

            # ---- loads: leading chunks as separate small tiles so the
            # projection stream starts as early as possible
            wkv_t, hs_t = [], []
            hs_groups = [[0, 1], [2, 3], [4, 5], [6, 7]]
            wkv_groups = [[0, 1], [2, 3], [4, 5], [6, 7]]
            wkv_of = {}
            hs_of = {}
            for i, grp in enumerate(wkv_groups):
                t_ = sb.tile([P, len(grp) * 2 * EH], BF16, tag=f"wkv{i}", name=f"wkv{i}")
                wkv_t.append(t_)
                for jj, c in enumerate(grp):
                    wkv_of[c] = (i, jj)
            for i, grp in enumerate(hs_groups):
                t_ = sb.tile([P, len(grp) * T], BF16, tag=f"hs{i}", name=f"hs{i}")
                hs_t.append(t_)
                for jj, c in enumerate(grp):
                    hs_of[c] = (i, jj)

            def dma_grp(tile_, dram_ap_3d, grp, width):
                nc.sync.dma_start(
                    tile_[:].rearrange("p (c n) -> p c n", c=len(grp)),
                    dram_ap_3d[:, grp[0]:grp[-1] + 1, :],
                )

            wkv3 = wkvt.ap().rearrange("(c p) n -> p c n", p=P)
            hs3 = hsT.ap().rearrange("(c p) t -> p c t", p=P)
            # issue order interleaves hs/wkv in consumption order
            order = [
                (wkv_t[0], wkv3, wkv_groups[0]),
                (hs_t[0], hs3, hs_groups[0]),
                (hs_t[1], hs3, hs_groups[1]),
                (wkv_t[1], wkv3, wkv_groups[1]),
                (hs_t[2], hs3, hs_groups[2]),
                (wkv_t[2], wkv3, wkv_groups[2]),
                (hs_t[3], hs3, hs_groups[3]),
                (wkv_t[3], wkv3, wkv_groups[3]),
            ]
            for t_, ap3, grp in order:
                dma_grp(t_, ap3, grp, 0)

            bkv_sb = sb.tile([P, 2 * EH + 16], F32, tag="bkv")
            nc.sync.dma_start(bkv_sb[:], bkvb[:, :])
            bq_sb = bkv_sb
            wq_big = sb.tile([P, KC * EH], BF16, tag="wq")
            nc.sync.dma_start(
                wq_big[:].rearrange("p (c n) -> p c n", c=KC),
                wqt.ap().rearrange("(c p) n -> p c n", p=P),
            )
            wo_sb = [sb.tile([P, E], BF16, tag=f"wo{c}", name=f"wo{c}") for c in range(2)]
            for c in range(2):
                nc.sync.dma_start(wo_sb[c][:], wot[c * P:(c + 1) * P, :])

            def hs_c(c):
                i, jj = hs_of[c]
                return hs_t[i][:, jj * T:(jj + 1) * T]

            def wkv_c(c):
                i, jj = wkv_of[c]
                return wkv_t[i][:, jj * 2 * EH:(jj + 1) * 2 * EH]

            # ---- fused k|v projection: [128 tokens, k(4 heads)|v(4 heads)]
            kv_sb = [
                sb.tile([P, 2 * EH], BF16, tag=f"kv{tt}", name=f"kv{tt}")
                for tt in range(TTC)
            ]
            for tt in range(TTC):
                ps = psA.tile([P, 512], F32, tag="psA")
                for c in range(KC):
                    nc.tensor.matmul(
                        ps[:],
                        hs_c(c)[:, tt * P:(tt + 1) * P],
                        wkv_c(c),
                        start=(c == 0),
                        stop=(c == KC - 1),
                    )
                nc.vector.tensor_add(kv_sb[tt][:], ps[:], bkv_sb[:, 0:2 * EH])

            # ---- per-head K^T V (full batch, local: no collective)
            # head pairs stacked on partitions: head 2j+hh at rows hh*64,
            # cols j*64 — bases line up with qT slices in the Q@KTV matmul.
            # pair-packed V^T K: ONE MM per (pair, chunk) — lhsT = [v_A|v_B]
            # (M=128) against rhs = [k_A|k_B] (N=128); the useful diagonal
            # [64,64] blocks (VTK_h = KTV_h^T) land in a zeroed block-diagonal
            # [128,128] tile per pair.
            vtk_bd = [
                sb.tile([P, P], BF16, tag=f"vtk_bd{j}", name=f"vtk_bd{j}")
                for j in range(HPC // 2)
            ]
            for j in range(HPC // 2):
                nc.gpsimd.memset(vtk_bd[j][:], 0.0)
            for j in range(HPC // 2):
                ps = psB.tile([P, 2 * DH], F32, tag="psB")
                for tt in range(TTC):
                    nc.tensor.matmul(
                        ps[:],
                        kv_sb[tt][:, EH + 2 * j * DH:EH + (2 * j + 2) * DH],
                        kv_sb[tt][:, 2 * j * DH:(2 * j + 2) * DH],
                        start=(tt == 0),
                        stop=(tt == TTC - 1),
                    )
                nc.vector.tensor_copy(vtk_bd[j][0:DH, 0:DH], ps[0:DH, 0:DH])
                nc.vector.tensor_copy(
                    vtk_bd[j][DH:2 * DH, DH:2 * DH], ps[DH:2 * DH, DH:2 * DH]
                )
            # fold the out-projection through KTV once per pair:
            # M_j = blockdiag(KTV_A, KTV_B) @ WoT_pair  (out^T = sum_j M_j^T qT_j)
            m_sb = [
                sb.tile([P, E], BF16, tag=f"m{j}", name=f"m{j}")
                for j in range(HPC // 2)
            ]
            for j in range(HPC // 2):
                for half in range(2):
                    ps = psB.tile([P, 512], F32, tag="psB")
                    nc.tensor.matmul(
                        ps[:],
                        vtk_bd[j][:],
                        wo_sb[j][:, half * 512:(half + 1) * 512],
                        start=True,
                        stop=True,
                    )
                    nc.vector.tensor_copy(
                        m_sb[j][:, half * 512:(half + 1) * 512], ps[:]
                    )

            # ---- qT projection [e_out 256, tokens], bias + 1/64 folded
            q_sb = [
                sb.tile([P, T], BF16, tag=f"q{m}", name=f"q{m}")
                for m in range(EH // P)
            ]
            for m in range(EH // P):
                for tg in range(TG):
                    ps = psA.tile([P, 512], F32, tag="psA")
                    for c in range(KC):
                        nc.tensor.matmul(
                            ps[:],
                            wq_big[:, c * EH + m * P:c * EH + (m + 1) * P],
                            hs_c(c)[:, tg * 512:(tg + 1) * 512],
                            start=(c == 0),
                            stop=(c == KC - 1),
                        )
                    nc.scalar.activation(
                        q_sb[m][:, tg * 512:(tg + 1) * 512], ps[:], Ident,
                        bias=bkv_sb[:, 2 * EH + m:2 * EH + m + 1], scale=1.0 / 64.0,
                    )

            # ---- partial out^T = sum_j M_j^T @ qT_j (no bias: host adds bo)
            for m in range(KC):
                o_stage = stg.tile([P, T], out_dt, tag="ostg")
                for tg in range(TG):
                    ps = psA.tile([P, 512], F32, tag="psA")
                    for c in range(2):
                        nc.tensor.matmul(
                            ps[:],
                            m_sb[c][:, m * P:(m + 1) * P],
                            q_sb[c][:, tg * 512:(tg + 1) * 512],
                            start=(c == 0),
                            stop=(c == 1),
                        )
                    if tg % 2 == 0:
                        nc.vector.tensor_copy(o_stage[:, tg * 512:(tg + 1) * 512], ps[:])
                    else:
                        nc.scalar.copy(o_stage[:, tg * 512:(tg + 1) * 512], ps[:])
                nc.sync.dma_start(outT[m * P:(m + 1) * P, :], o_stage[:])

    nc.compile()
    return nc


_NC_CACHE = None


def _get_nc():
    global _NC_CACHE
    if _NC_CACHE is None:
        _install_axon_profile_hook()
        _NC_CACHE = build()
    return _NC_CACHE


def bias_tile(bkv, bq_slice):
    """[512] kv-bias + [256] scaled q-bias -> one [128, 528] f32 DMA tile:
    cols 0..512 = kv bias broadcast rows, cols 512..514 = bq/64 chunks."""
    t = np.zeros((P, 2 * EH + 16), np.float32)
    t[:, 0:2 * EH] = bkv
    t[:, 2 * EH:2 * EH + 2] = bq_slice.reshape(2, P).T
    return t


def make_in_maps(hidden_states, Wq, bq, Wk, bk, Wv, bv, Wo, bo):
    f32 = np.float32
    hs = np.asarray(hidden_states, f32)
    WqT = np.asarray(Wq, f32).T    # [e_in, e_out]
    WkT = np.asarray(Wk, f32).T
    WvT = np.asarray(Wv, f32).T
    WoT = np.asarray(Wo, f32).T
    bq64 = np.asarray(bq, f32) / 64.0
    bk = np.asarray(bk, f32)
    bv = np.asarray(bv, f32)

    hsT_b = [
        np.ascontiguousarray(hs[b].T).astype(NPBF16) for b in range(B)
    ]
    in_maps = []
    for i in range(NC):
        g, r = divmod(i, HPC)
        sl = slice(r * EH, (r + 1) * EH)
        wkvt = np.concatenate([WkT[:, sl], WvT[:, sl]], axis=1)
        bkv = np.concatenate([bk[sl], bv[sl]])
        in_maps.append({
            "hsT": hsT_b[g],
            "wkvt": np.ascontiguousarray(wkvt).astype(NPBF16),
            "wqt": np.ascontiguousarray(WqT[:, sl]).astype(NPBF16),
            "wot": np.ascontiguousarray(WoT[sl, :]).astype(NPBF16),
            "bkvb": bias_tile(bkv, bq64[sl]),
        })
    return in_maps


def run(inputs, trace=False, **kw):
    """Run on 8 NeuronCores; returns (full_output [B,T,E] f32, BassKernelResults)."""
    nc = _get_nc()
    in_maps = make_in_maps(**inputs)
    try:
        res = run_bass_kernel_spmd(nc, in_maps, list(range(NC)), trace=trace, **kw)
    except Exception:
        # rare transient NRT_EXEC_UNIT_UNRECOVERABLE — one retry usually lands
        res = run_bass_kernel_spmd(nc, in_maps, list(range(NC)), trace=trace, **kw)
    bo = np.asarray(inputs["bo"], np.float32)
    out = np.empty((B, T, E), np.float32)
    for g in range(B):
        acc = res.results[g * HPC]["outT"].astype(np.float32)
        for r in range(1, HPC):
            acc = acc + res.results[g * HPC + r]["outT"].astype(np.float32)
        out[g] = acc.T + bo
    return out, res


def kernel(**inputs):
    out, _ = run(inputs, trace=False)
    return out
